# revision 1
# baseline (speedup 1.0000x reference)
"""LongLlama attention (B=1, S=4096, HID=2048, 16 heads) on 8 TRN2 NeuronCores.

Sharding: tensor-parallel over heads (2 heads/core). Each core computes its
heads' Q/K/V projections, RoPE, causal attention, and the partial output
projection attn_out_h @ Wo[:, h_slice].T; the host sums the 8 partials
(the TP all-reduce, done at unshard time).

Device layout: transposed-activation space. Host passes hidden^T (bf16),
transposed weight slices, RoPE tables cos^T/sin^T, rotate_half as a +-1
permutation matrix R (so the partition-dim rotate becomes a small matmul),
and exp(mask) tiles for diagonal blocks. Scores are computed directly in
S^T[kv, q] layout: softmax denominators come from a ones-vector matmul and
P@V needs no transposes. Blocks whose exp(mask) is identically 0 are skipped
(causal upper triangle); identically-1 blocks skip the mask multiply. This
is mathematically exact for any additive mask: exp(s+m) = exp(s)*exp(m).
"""

import sys

sys.path.insert(0, "/opt/trn_rl_repo")

import numpy as np
import ml_dtypes

NUM_HEADS = 16
N_CORES = 8
HID = 2048
D = HID // NUM_HEADS  # 128
HPC = NUM_HEADS // N_CORES  # 2 heads per core
DPC = D * HPC  # 256 output channels per core
QSUP = 512  # q columns processed per attention pass
KBLK = 128  # kv block (matmul contraction)
P = 128

BF16 = ml_dtypes.bfloat16

_cache = {}


def _classify_mask(mask, S):
    """Per (q-super, kv-block) classification from exp(mask):
    's' all-zero (skip), 'p' all-one (plain), 'm' general (multiply).
    Returns (classes, masked_tiles[kv,q] bf16)."""
    em = np.exp(mask.astype(np.float32))
    nsup = S // QSUP
    nkv = S // KBLK
    classes = []
    tiles = []
    index = {}
    for i in range(nsup):
        row = []
        for j in range(nkv):
            t = em[i * QSUP:(i + 1) * QSUP, j * KBLK:(j + 1) * KBLK]
            if not np.any(t):
                row.append('s')
            elif np.all(t == 1.0):
                row.append('p')
            else:
                row.append('m')
                index[(i, j)] = len(tiles)
                tiles.append(np.ascontiguousarray(t.T).astype(BF16))
        classes.append(tuple(row))
    if tiles:
        em_stack = np.stack(tiles)
    else:
        em_stack = np.zeros((1, KBLK, QSUP), dtype=BF16)
    return tuple(classes), em_stack, index


def _build(S, classes, em_index, n_em):
    import concourse.tile as tile
    from concourse import bacc, mybir

    f32 = mybir.dt.float32
    bf16 = mybir.dt.bfloat16
    Exp = mybir.ActivationFunctionType.Exp

    NSUP = S // QSUP
    NKV = S // KBLK
    HO = HID // P  # 16 contraction subtiles

    nc = bacc.Bacc("TRN2", target_bir_lowering=False, debug=False,
                   num_devices=N_CORES)

    hidT = nc.dram_tensor("hidT", [HID, S], bf16, kind="ExternalInput").ap()
    cosT_d = nc.dram_tensor("cosT", [D, S], bf16, kind="ExternalInput").ap()
    sinT_d = nc.dram_tensor("sinT", [D, S], bf16, kind="ExternalInput").ap()
    wqT_d = nc.dram_tensor("wqT", [HID, DPC], bf16, kind="ExternalInput").ap()
    wkT_d = nc.dram_tensor("wkT", [HID, DPC], bf16, kind="ExternalInput").ap()
    wvT_d = nc.dram_tensor("wvT", [HID, DPC], bf16, kind="ExternalInput").ap()
    woT_d = nc.dram_tensor("woT", [DPC, HID], bf16, kind="ExternalInput").ap()
    r_d = nc.dram_tensor("rmat", [D, D], bf16, kind="ExternalInput").ap()
    em_d = nc.dram_tensor("emask", [n_em, KBLK, QSUP], bf16,
                          kind="ExternalInput").ap()
    out_d = nc.dram_tensor("outp", [S, HID], f32, kind="ExternalOutput").ap()

    hidT_r = hidT.rearrange("(ho hi) s -> hi ho s", hi=P)
    wqT_r = wqT_d.rearrange("(ho hi) d -> hi ho d", hi=P)
    wkT_r = wkT_d.rearrange("(ho hi) d -> hi ho d", hi=P)
    wvT_r = wvT_d.rearrange("(ho hi) d -> hi ho d", hi=P)
    woT_r = woT_d.rearrange("(ho hi) e -> hi ho e", hi=P)

    SCALE = 1.0 / float(np.sqrt(np.float64(D)))

    with tile.TileContext(nc) as tc:
        with (
            tc.tile_pool(name="const", bufs=1) as const,
            tc.tile_pool(name="resid", bufs=1) as resid,
            tc.tile_pool(name="ht", bufs=2) as ht_pool,
            tc.tile_pool(name="rope", bufs=2) as rope,
            tc.tile_pool(name="ptp", bufs=4) as ptp,
            tc.tile_pool(name="otp", bufs=2) as otp,
            tc.tile_pool(name="smal", bufs=2) as smal,
            tc.tile_pool(name="outs", bufs=3) as outs,
            tc.tile_pool(name="em", bufs=8) as em_pool,
            tc.tile_pool(name="ps_qk", bufs=2, space="PSUM") as ps_qk,
            tc.tile_pool(name="ps_v", bufs=1, space="PSUM") as ps_v,
            tc.tile_pool(name="ps_st", bufs=2, space="PSUM") as ps_st,
            tc.tile_pool(name="ps_ot", bufs=1, space="PSUM") as ps_ot,
            tc.tile_pool(name="ps_l", bufs=1, space="PSUM") as ps_l,
            tc.tile_pool(name="ps_wo", bufs=1, space="PSUM") as ps_wo,
        ):
            ones_bf = const.tile([P, 1], bf16, tag="ones_bf")
            nc.any.memset(ones_bf, 1.0)
            ones_f = const.tile([1, P], f32, tag="ones_f")
            nc.any.memset(ones_f, 1.0)
            rt = const.tile([D, D], bf16, tag="rt")
            nc.sync.dma_start(rt, r_d)
            cosT = const.tile([D, S], bf16, tag="cosT")
            nc.sync.dma_start(cosT, cosT_d)
            sinT = const.tile([D, S], bf16, tag="sinT")
            nc.sync.dma_start(sinT, sinT_d)
            wqT = const.tile([P, HO, DPC], bf16, tag="wqT")
            nc.sync.dma_start(wqT, wqT_r)
            wkT = const.tile([P, HO, DPC], bf16, tag="wkT")
            nc.sync.dma_start(wkT, wkT_r)
            wvT = const.tile([P, HO, DPC], bf16, tag="wvT")
            nc.sync.dma_start(wvT, wvT_r)
            woT = const.tile([P, HPC, HID], bf16, tag="woT")
            nc.sync.dma_start(woT, woT_r)

            QT = resid.tile([D, HPC, S], bf16, tag="QT")
            KT = resid.tile([D, HPC, S], bf16, tag="KT")
            Vr = resid.tile([P, NKV, DPC], bf16, tag="Vr")

            for i in range(NSUP):
                qsl = slice(i * QSUP, (i + 1) * QSUP)

                ht = ht_pool.tile([P, HO, QSUP], bf16, tag="ht")
                nc.sync.dma_start(ht, hidT_r[:, :, qsl])

                # ---- Q/K projections + RoPE (per head) ----
                for w_t, dest in ((wqT, QT), (wkT, KT)):
                    for h in range(HPC):
                        pp = ps_qk.tile([P, QSUP], f32, tag="qk")
                        for ho in range(HO):
                            nc.tensor.matmul(
                                pp, lhsT=w_t[:, ho, h * D:(h + 1) * D],
                                rhs=ht[:, ho, :],
                                start=(ho == 0), stop=(ho == HO - 1))
                        qbf = rope.tile([P, QSUP], bf16, tag="qbf")
                        nc.scalar.copy(qbf, pp)
                        rp = ps_qk.tile([P, QSUP], f32, tag="qk")
                        nc.tensor.matmul(rp, lhsT=rt, rhs=qbf,
                                         start=True, stop=True)
                        rbf = rope.tile([P, QSUP], bf16, tag="rbf")
                        nc.scalar.copy(rbf, rp)
                        t1 = rope.tile([P, QSUP], bf16, tag="t1")
                        nc.vector.tensor_mul(t1, qbf, cosT[:, qsl])
                        t2 = rope.tile([P, QSUP], bf16, tag="t2")
                        nc.vector.tensor_mul(t2, rbf, sinT[:, qsl])
                        nc.vector.tensor_add(dest[:, h, qsl], t1, t2)

                # ---- V projection ----
                for sb in range(QSUP // P):
                    vp = ps_v.tile([P, DPC], f32, tag="v")
                    for ho in range(HO):
                        nc.tensor.matmul(
                            vp, lhsT=ht[:, ho, sb * P:(sb + 1) * P],
                            rhs=wvT[:, ho, :],
                            start=(ho == 0), stop=(ho == HO - 1))
                    nc.scalar.copy(Vr[:, i * (QSUP // P) + sb, :], vp)

                # ---- masked-block exp(mask) tiles for this super ----
                em_ts = {}
                for j in range(NKV):
                    if classes[i][j] == 'm':
                        t = em_pool.tile([KBLK, QSUP], bf16, tag="em")
                        nc.sync.dma_start(t, em_d[em_index[(i, j)]])
                        em_ts[j] = t

                # ---- attention (per head) ----
                ot_sb = otp.tile([P, HPC, QSUP], bf16, tag="ot_sb")
                for h in range(HPC):
                    kvs = [j for j in range(NKV) if classes[i][j] != 's']
                    nblk = len(kvs)
                    ot_ps = ps_ot.tile([P, QSUP], f32, tag="ot")
                    l_ps = ps_l.tile([1, QSUP], f32, tag="l")

                    def emit_st(j):
                        stp = ps_st.tile([P, QSUP], f32, tag="st")
                        nc.tensor.matmul(
                            stp, lhsT=KT[:, h, j * KBLK:(j + 1) * KBLK],
                            rhs=QT[:, h, qsl], start=True, stop=True)
                        return stp

                    prev = emit_st(kvs[0])
                    for idx, j in enumerate(kvs):
                        nxt = emit_st(kvs[idx + 1]) if idx + 1 < nblk else None
                        pt = ptp.tile([KBLK, QSUP], bf16, tag="pt")
                        nc.scalar.activation(pt, prev, Exp, scale=SCALE)
                        if classes[i][j] == 'm':
                            nc.vector.tensor_mul(pt, pt, em_ts[j])
                        nc.tensor.matmul(
                            ot_ps, lhsT=Vr[:, j, h * D:(h + 1) * D], rhs=pt,
                            start=(idx == 0), stop=(idx == nblk - 1))
                        nc.tensor.matmul(
                            l_ps, lhsT=ones_bf, rhs=pt,
                            start=(idx == 0), stop=(idx == nblk - 1))
                        prev = nxt

                    # normalize: ot_sb[:,h,:] = ot_ps * broadcast(1/l)
                    linv = smal.tile([1, QSUP], f32, tag="linv")
                    nc.vector.reciprocal(linv, l_ps)
                    bc = ps_st.tile([P, QSUP], f32, tag="st")
                    nc.tensor.matmul(bc, lhsT=ones_f, rhs=linv,
                                     start=True, stop=True)
                    bcs = smal.tile([P, QSUP], f32, tag="bcs")
                    nc.scalar.copy(bcs, bc)
                    nc.vector.tensor_mul(ot_sb[:, h, :], ot_ps, bcs)

                # ---- output projection (partial over this core's heads) ----
                for sb in range(QSUP // P):
                    srow = (i * (QSUP // P) + sb) * P
                    for ec in range(HID // QSUP):
                        wo = ps_wo.tile([P, QSUP], f32, tag="wo")
                        for h in range(HPC):
                            nc.tensor.matmul(
                                wo, lhsT=ot_sb[:, h, sb * P:(sb + 1) * P],
                                rhs=woT[:, h, ec * QSUP:(ec + 1) * QSUP],
                                start=(h == 0), stop=(h == HPC - 1))
                        ob = outs.tile([P, QSUP], f32, tag="ob")
                        nc.scalar.copy(ob, wo)
                        nc.sync.dma_start(
                            out_d[srow:srow + P, ec * QSUP:(ec + 1) * QSUP],
                            ob)

    nc.compile()
    return nc


def _prepare(hidden_states, attention_mask, position_ids, Wq, Wk, Wv, Wo):
    """Host-side sharding prep. Returns (nc, in_maps)."""
    B, S, hid = hidden_states.shape
    assert B == 1 and hid == HID

    classes, em_stack, em_index = _classify_mask(
        np.asarray(attention_mask)[0, 0], S)

    key = (S, classes)
    if key not in _cache:
        _cache[key] = _build(S, classes, em_index, em_stack.shape[0])
    nc = _cache[key]

    hidT = np.ascontiguousarray(
        np.asarray(hidden_states)[0].T).astype(BF16)  # [HID, S]

    # RoPE tables, exactly as the reference computes them (fp32)
    pos = np.asarray(position_ids)[0]
    rel = (pos - pos.min()).astype(np.int64)
    inv_freq = 1.0 / (10000.0 ** (np.arange(0, D, 2, dtype=np.float32) / D))
    t = np.arange(S, dtype=np.float32)
    freqs = t[:, None] * inv_freq[None, :]
    emb = np.concatenate([freqs, freqs], axis=-1)  # [S, D]
    cos_t = np.cos(emb).astype(np.float32)[rel]  # [S, D]
    sin_t = np.sin(emb).astype(np.float32)[rel]
    cosT = np.ascontiguousarray(cos_t.T).astype(BF16)
    sinT = np.ascontiguousarray(sin_t.T).astype(BF16)

    # rotate_half as matrix: rot = R.T @ q  (rot[d']=-q[d'+64] / q[d'-64])
    R = np.zeros((D, D), dtype=np.float32)
    for dp in range(D // 2):
        R[dp + D // 2, dp] = -1.0
    for dp in range(D // 2, D):
        R[dp - D // 2, dp] = 1.0
    R = R.astype(BF16)

    Wq = np.asarray(Wq)
    Wk = np.asarray(Wk)
    Wv = np.asarray(Wv)
    Wo = np.asarray(Wo)

    in_maps = []
    for c in range(N_CORES):
        rs = slice(c * DPC, (c + 1) * DPC)
        in_maps.append({
            "hidT": hidT,
            "cosT": cosT,
            "sinT": sinT,
            "wqT": np.ascontiguousarray(Wq[rs, :].T).astype(BF16),
            "wkT": np.ascontiguousarray(Wk[rs, :].T).astype(BF16),
            "wvT": np.ascontiguousarray(Wv[rs, :].T).astype(BF16),
            "woT": np.ascontiguousarray(Wo[:, rs].T).astype(BF16),
            "rmat": R,
            "emask": em_stack,
        })
    return nc, in_maps


def kernel(hidden_states, attention_mask, position_ids, Wq, Wk, Wv, Wo):
    from concourse.bass_utils import run_bass_kernel_spmd

    nc, in_maps = _prepare(hidden_states, attention_mask, position_ids,
                           Wq, Wk, Wv, Wo)
    res = run_bass_kernel_spmd(nc, in_maps, core_ids=list(range(N_CORES)))
    B, S, hid = hidden_states.shape
    out = np.zeros((S, HID), dtype=np.float32)
    for c in range(N_CORES):
        out += res.results[c]["outp"]
    return out.reshape(B, S, HID).astype(np.float32)
